# revision 9
# baseline (speedup 1.0000x reference)
"""Trainium2 Bass kernel for nn_ModelNew_3556232922055 (dense_cnn).

Semantics (per image):
  y8[j]    = conv2d_valid(x, weight[:8]) + bias[:8]          (8,126,126)
  acc[co]  = max over (ci,kh,kw) of 2*W[co,ci,kh,kw]*y8[ci,h+kh,w+kw]
             (out-of-range taps excluded at the bottom/right borders)
  out      = min over co of acc                              (1,126,126)

Sharding: data-parallel over batch, 1 image per NeuronCore (8 cores).

Device mapping per core (v2 — DVE-centric, no GPSIMD):
  - host-built im2col X72 [72, 16128] bf16, streamed in 2 DMA halves
  - conv as k=72 bf16 matmuls (504-col chunks) -> PSUM -> ACT evac (+bias)
    into Y8 [128, 16128] bf16 where partition p = ci*16 + r holds y8[ci]
  - step 2 in 3 row-bands; per tap and co-half: product into scratch
    (DVE tensor_scalar at 4x for aligned taps, ACT mul for the rest),
    then DVE tensor_tensor max-accumulate at 2x
    (tap 0 initializes acc via tensor_scalar mult)
  - per band reduce: PE-transpose 128x128 chunks -> ACT evac -> DVE
    max-fold over ci, min over 32 co -> OUT[w,h]; finally PE transpose ->
    DMA out (126,126) f32.
"""

import numpy as np
from contextlib import ExitStack

import ml_dtypes

import concourse.bass as bass
import concourse.tile as tile
from concourse import bacc, mybir
from concourse import masks
from concourse.bass_utils import run_bass_kernel_spmd

F32 = mybir.dt.float32
BF16 = mybir.dt.bfloat16

DT_Y = BF16    # y8 replicas
DT_ACC = BF16  # pacc accumulators

H = W = 128
CIN = 8
COUT = 32
K = 3
OH = OW = 126
NPIX = H * OH          # 16128 flat pixels (h*128+w), h<126
NCORES = 8
CHUNK = 504            # conv free-dim chunk (<=512, 32*504=16128)
GROUP = 16             # reduce-phase chunks (=output rows) per psum group
BANDS = [(0, 42), (42, 84), (84, 126)]

TAPS = [(kh, kw) for kh in range(K) for kw in range(K)]


def _r3(t, h0, nh, w0, nw):
    """3D region view [128, nh, nw] of a [128, NPIX] tile at rows h0, cols w0."""
    return t[:].rearrange("p (h w) -> p h w", w=W)[:, h0 : h0 + nh, w0 : w0 + nw]


def build_program():
    nc = bacc.Bacc()

    # x72: host-built im2col, x72[(kh*3+kw)*8+ci, pix] = x[ci, pix+kh*128+kw]
    x_d = nc.declare_dram_parameter("x72", [72, NPIX], BF16, isOutput=False)
    # consts: [:, 0:18] scal, [:, 18:19] bias128
    c_d = nc.declare_dram_parameter("consts", [128, 19], F32, isOutput=False)
    w_d = nc.declare_dram_parameter("w1", [72, 128], BF16, isOutput=False)
    out_d = nc.declare_dram_parameter("out", [OH, OW], F32, isOutput=True)

    with ExitStack() as ctx:
        tc = ctx.enter_context(tile.TileContext(nc))

        consts = ctx.enter_context(tc.tile_pool(name="consts", bufs=1))
        big = ctx.enter_context(tc.tile_pool(name="big", bufs=1))

        constst = consts.tile([128, 19], F32)
        nc.sync.dma_start(constst[:], c_d[:])
        scalt = constst[:, 0:18]
        biast = constst[:, 18:19]
        w1t = consts.tile([72, 128], BF16)
        nc.sync.dma_start(w1t[:], w_d[:])
        ident = consts.tile([128, 128], DT_ACC)
        masks.make_identity(nc, ident[:])
        ident_f32 = consts.tile([128, 128], F32)
        masks.make_identity(nc, ident_f32[:])

        y8 = big.tile([128, NPIX], DT_Y)
        paccs = []
        for half in range(2):
            pacc = big.tile([128, NPIX], DT_ACC, tag=f"pacc{half}")
            paccs.append(pacc)
        outt = big.tile([128, OH], F32)  # OUT[w, h]

        redpool = ctx.enter_context(tc.tile_pool(name="redpool", bufs=2))
        ppa = ctx.enter_context(tc.tile_pool(name="ppa", bufs=3))
        ppv = ctx.enter_context(tc.tile_pool(name="ppv", bufs=2))
        xp = ctx.enter_context(tc.tile_pool(name="xp", bufs=1))
        psum = ctx.enter_context(tc.tile_pool(name="psum", bufs=2, space="PSUM"))
        psred = ctx.enter_context(tc.tile_pool(name="psred", bufs=1, space="PSUM"))

        # --- load + conv: y8[p = ci*16+r] = y8[ci], bf16 ---
        # x72 streamed in 8 small pieces so the first matmul starts early;
        # the first 12 chunks (band-0 rows) alternate evac between ACT and
        # DVE (both idle in the prefix) to cut the serial prefix.
        PIECE = 4 * CHUNK
        for pc in range(8):
            xh = xp.tile([72, PIECE], BF16, tag="xh")
            nc.sync.dma_start(out=xh[:], in_=x_d[:, pc * PIECE : (pc + 1) * PIECE])
            for c in range(4):
                ci_ = pc * 4 + c
                n0 = ci_ * CHUNK
                ps = psum.tile([128, CHUNK], F32, tag="convps")
                nc.tensor.matmul(
                    ps[:], lhsT=w1t[:], rhs=xh[:, c * CHUNK : (c + 1) * CHUNK],
                    start=True, stop=True,
                )
                if ci_ < 12 and ci_ % 2 == 1:
                    nc.vector.tensor_scalar(
                        y8[:, n0 : n0 + CHUNK], ps[:], biast, None,
                        mybir.AluOpType.add,
                    )
                else:
                    nc.scalar.activation(
                        y8[:, n0 : n0 + CHUNK], ps[:],
                        mybir.ActivationFunctionType.Identity,
                        bias=biast, scale=1.0,
                    )

        mx = mybir.AluOpType.max
        mn = mybir.AluOpType.min
        mult = mybir.AluOpType.mult

        def reduce_group(c0, gc):
            """Fold chunks [c0, c0+gc) of both pacc halves into outt[:, c0:c0+gc]."""
            ps_a = psred.tile([128, gc * 128], DT_ACC, tag="ps_a")
            ps_b = psred.tile([128, gc * 128], DT_ACC, tag="ps_b")
            pst = [ps_a, ps_b]
            for half in range(2):
                for j in range(gc):
                    nc.tensor.transpose(
                        pst[half][:, j * 128 : (j + 1) * 128],
                        paccs[half][:, (c0 + j) * 128 : (c0 + j + 1) * 128],
                        ident[:],
                    )
            pt = redpool.tile([128, 2 * GROUP * 128], DT_ACC, tag="PT")
            for half in range(2):
                nc.scalar.copy(
                    pt[:, half * gc * 128 : (half + 1) * gc * 128], pst[half][:]
                )
            # pt layout: [p][s=2][c=gc][ci=8][co=16] (valid region only)
            v = pt[:, 0 : 2 * gc * 128].rearrange(
                "p (s c ci co) -> p s c ci co", s=2, c=gc, ci=8
            )
            nc.vector.tensor_tensor(
                v[:, :, :, 0:4, :], v[:, :, :, 0:4, :], v[:, :, :, 4:8, :], mx)
            nc.vector.tensor_tensor(
                v[:, :, :, 0:2, :], v[:, :, :, 0:2, :], v[:, :, :, 2:4, :], mx)
            nc.vector.tensor_tensor(
                v[:, :, :, 0:1, :], v[:, :, :, 0:1, :], v[:, :, :, 1:2, :], mx)
            # min(A-half, B-half) -> co 0..15 vs 16..31
            nc.vector.tensor_tensor(
                v[:, 0:1, :, 0:1, :], v[:, 0:1, :, 0:1, :], v[:, 1:2, :, 0:1, :], mn)
            w2 = v[:, 0, :, 0, :]  # [p, c, co16]
            nc.vector.tensor_reduce(
                outt[:, c0 : c0 + gc], w2, mybir.AxisListType.X, mn)

        # --- step 2, banded; reduce band k while band k+1 computes ---
        for h0, h1 in BANDS:
            bh = h1 - h0
            # tap 0 initializes acc over the full band (incl. junk cols 126/127)
            for half in range(2):
                nc.vector.tensor_scalar(
                    _r3(paccs[half], h0, bh, 0, W), _r3(y8, h0, bh, 0, W),
                    scalt[:, half * 9 : half * 9 + 1], None, mult,
                )
            # taps with kw=1 read y8 at a 2-byte-misaligned column offset,
            # which drops DVE tensor_scalar out of its packed modes -> ACT.
            # ACT additionally takes taps 2,5 (kw=2) to balance engine load.
            for t in range(1, 9):
                kh, kw = TAPS[t]
                nh = min(h1, OH - kh) - h0
                nw = OW - kw
                prods = []
                for half in range(2):
                    on_act = t in (1, 2, 4, 5, 7) or (t == 8 and half == 0)
                    pool = ppa if on_act else ppv
                    p = pool.tile([128, 42 * W], DT_ACC, tag="PA" if on_act else "PV")
                    p3 = p[:].rearrange("p (h w) -> p h w", w=W)[:, 0:nh, 0:nw]
                    src3 = _r3(y8, h0 + kh, nh, kw, nw)
                    sc = scalt[:, half * 9 + t : half * 9 + t + 1]
                    if on_act:
                        nc.scalar.mul(p3, src3, sc)
                    else:
                        nc.vector.tensor_scalar(p3, src3, sc, None, mult)
                    prods.append(p3)
                for half in range(2):
                    acc3 = _r3(paccs[half], h0, nh, 0, nw)
                    nc.vector.tensor_tensor(acc3, acc3, prods[half], mx)
            g = 9 if h1 == OH else GROUP  # finer groups in the last band
            for c0 in range(h0, h1, g):
                reduce_group(c0, min(g, h1 - c0))

        # transpose OUT[w,h] -> [h,w] and write out
        pso = psred.tile([128, 128], F32, tag="pso")
        nc.tensor.transpose(pso[0:OH, :], outt[:, 0:OH], ident_f32[:])
        res = consts.tile([128, 128], F32)
        nc.scalar.copy(res[0:OH, :], pso[0:OH, :])
        nc.sync.dma_start(out_d[:, :], res[0:OH, 0:OW])

    nc.compile()
    return nc


def host_tiles(weight, bias):
    weight = np.asarray(weight, np.float32)
    bias = np.asarray(bias, np.float32)
    w1rep = np.zeros((72, 128), np.float32)
    for kh in range(K):
        for kw in range(K):
            for ci_in in range(CIN):
                t = (kh * K + kw) * CIN + ci_in
                for ci_out in range(CIN):
                    w1rep[t, ci_out * 16 : ci_out * 16 + 16] = weight[
                        ci_out, ci_in, kh, kw
                    ]
    bias128 = np.repeat(bias[:CIN], 16).astype(np.float32).reshape(128, 1)
    scal = np.zeros((128, 18), np.float32)
    for p in range(128):
        ci = p // 16
        co_lo = p % 16
        for half in range(2):
            co = co_lo + 16 * half
            for t, (kh, kw) in enumerate(TAPS):
                scal[p, half * 9 + t] = 2.0 * weight[co, ci, kh, kw]
    consts = np.zeros((128, 19), np.float32)
    consts[:, 0:18] = scal
    consts[:, 18:19] = bias128
    return consts, w1rep.astype(ml_dtypes.bfloat16)


def im2col_host(xb):
    """xb: (8,128,128) f32 -> (72, NPIX) bf16 with junk tail cols zeroed."""
    x72 = np.zeros((72, NPIX), np.float32)
    L = NPIX - 2
    flat = xb.reshape(-1)
    for kh in range(K):
        for kw in range(K):
            for ci in range(CIN):
                t = (kh * K + kw) * CIN + ci
                off = kh * W + kw
                x72[t, :L] = flat[ci * H * W + off : ci * H * W + off + L]
    return x72.astype(ml_dtypes.bfloat16)


_CACHE = {}


def _get_program():
    if "nc" not in _CACHE:
        _CACHE["nc"] = build_program()
    return _CACHE["nc"]


def run_spmd(x, weight, bias, **kw):
    x = np.ascontiguousarray(np.asarray(x, np.float32))
    consts, w1rep = host_tiles(weight, bias)
    nc = _get_program()
    in_maps = [
        {"x72": im2col_host(x[b]), "consts": consts, "w1": w1rep}
        for b in range(NCORES)
    ]
    res = run_bass_kernel_spmd(nc, in_maps, list(range(NCORES)), **kw)
    out = np.stack([res.results[b]["out"] for b in range(NCORES)])
    return out[:, None, :, :].astype(np.float32), res


def kernel(x, weight, bias):
    out, _ = run_spmd(x, weight, bias)
    return out


if __name__ == "__main__":
    rng = np.random.default_rng(0)
    x = rng.standard_normal((8, CIN, H, W), dtype=np.float32)
    wt = rng.uniform(-0.1, 0.1, (COUT, CIN, K, K)).astype(np.float32)
    bs = rng.uniform(-0.1, 0.1, COUT).astype(np.float32)
    print(kernel(x, wt, bs).shape)


# revision 20
# speedup vs baseline: 1.0429x; 1.0429x over previous
"""Trainium2 Bass kernel for nn_ModelNew_3556232922055 (dense_cnn).

Semantics (per image):
  y8[j]    = conv2d_valid(x, weight[:8]) + bias[:8]          (8,126,126)
  acc[co]  = max over (ci,kh,kw) of 2*W[co,ci,kh,kw]*y8[ci,h+kh,w+kw]
             (out-of-range taps excluded at the bottom/right borders)
  out      = min over co of acc                              (1,126,126)

Sharding: data-parallel over batch, 1 image per NeuronCore (8 cores).

Device mapping per core (v2 — DVE-centric, no GPSIMD):
  - host-built im2col X72 [72, 16128] bf16, streamed in 2 DMA halves
  - conv as k=72 bf16 matmuls (504-col chunks) -> PSUM -> ACT evac (+bias)
    into Y8 [128, 16128] bf16 where partition p = ci*16 + r holds y8[ci]
  - step 2 in 3 row-bands; per tap and co-half: product into scratch
    (DVE tensor_scalar at 4x for aligned taps, ACT mul for the rest),
    then DVE tensor_tensor max-accumulate at 2x
    (tap 0 initializes acc via tensor_scalar mult)
  - per band reduce: PE-transpose 128x128 chunks -> ACT evac -> DVE
    max-fold over ci, min over 32 co -> OUT[w,h]; finally PE transpose ->
    DMA out (126,126) f32.
"""

import numpy as np
from contextlib import ExitStack

import ml_dtypes

import concourse.bass as bass
import concourse.tile as tile
from concourse import bacc, mybir
from concourse import masks
from concourse.bass_utils import run_bass_kernel_spmd

F32 = mybir.dt.float32
BF16 = mybir.dt.bfloat16

DT_Y = BF16    # y8 replicas
DT_ACC = BF16  # pacc accumulators

H = W = 128
CIN = 8
COUT = 32
K = 3
OH = OW = 126
NPIX = H * OH          # 16128 flat pixels (h*128+w), h<126
NCORES = 8
CHUNK = 504            # conv free-dim chunk (<=512, 32*504=16128)
GROUP = 16             # max reduce-phase chunks per psum group
BANDS = [(0, 48), (48, 96), (96, 126)]

TAPS = [(kh, kw) for kh in range(K) for kw in range(K)]


def _r3(t, h0, nh, w0, nw):
    """3D region view [128, nh, nw] of a [128, NPIX] tile at rows h0, cols w0."""
    return t[:].rearrange("p (h w) -> p h w", w=W)[:, h0 : h0 + nh, w0 : w0 + nw]


def build_program():
    nc = bacc.Bacc()

    # x72: host-built im2col, x72[(kh*3+kw)*8+ci, pix] = x[ci, pix+kh*128+kw]
    x_d = nc.declare_dram_parameter("x72", [72, NPIX], BF16, isOutput=False)
    # consts: [:, 0:18] scal, [:, 18:19] bias128
    c_d = nc.declare_dram_parameter("consts", [128, 19], F32, isOutput=False)
    w_d = nc.declare_dram_parameter("w1", [72, 128], BF16, isOutput=False)
    out_d = nc.declare_dram_parameter("out", [OH, OW], F32, isOutput=True)

    with ExitStack() as ctx:
        tc = ctx.enter_context(tile.TileContext(nc))

        consts = ctx.enter_context(tc.tile_pool(name="consts", bufs=1))
        big = ctx.enter_context(tc.tile_pool(name="big", bufs=1))

        constst = consts.tile([128, 19], F32)
        nc.sync.dma_start(constst[:], c_d[:])
        scalt = constst[:, 0:18]
        biast = constst[:, 18:19]
        w1t = consts.tile([72, 128], BF16)
        nc.sync.dma_start(w1t[:], w_d[:])
        ident = consts.tile([128, 128], DT_ACC)
        masks.make_identity(nc, ident[:])
        ident_f32 = consts.tile([128, 128], F32)
        masks.make_identity(nc, ident_f32[:])

        y8 = big.tile([128, NPIX], DT_Y)
        paccs = []
        for half in range(2):
            pacc = big.tile([128, NPIX], DT_ACC, tag=f"pacc{half}")
            paccs.append(pacc)
        outt = big.tile([128, OH], F32)  # OUT[w, h]

        redpool = ctx.enter_context(tc.tile_pool(name="redpool", bufs=2))
        ppa = ctx.enter_context(tc.tile_pool(name="ppa", bufs=4))
        ppv = ctx.enter_context(tc.tile_pool(name="ppv", bufs=2))
        xp = ctx.enter_context(tc.tile_pool(name="xp", bufs=1))
        psum = ctx.enter_context(tc.tile_pool(name="psum", bufs=2, space="PSUM"))
        psred = ctx.enter_context(tc.tile_pool(name="psred", bufs=1, space="PSUM"))

        mx = mybir.AluOpType.max
        mn = mybir.AluOpType.min
        mult = mybir.AluOpType.mult

        # --- load + conv: y8[p = ci*16+r] = y8[ci], bf16 ---
        # x72 streamed in 8 small pieces so the first matmul starts early;
        # the first 12 chunks (band-0 rows) alternate evac between ACT and
        # DVE (both idle in the prefix) to cut the serial prefix.
        PIECE = 4 * CHUNK

        def conv_piece(pc):
            xh = xp.tile([72, PIECE], BF16, tag="xh")
            nc.sync.dma_start(out=xh[:], in_=x_d[:, pc * PIECE : (pc + 1) * PIECE])
            for c in range(4):
                ci_ = pc * 4 + c
                n0 = ci_ * CHUNK
                ps = psum.tile([128, CHUNK], F32, tag="convps")
                nc.tensor.matmul(
                    ps[:], lhsT=w1t[:], rhs=xh[:, c * CHUNK : (c + 1) * CHUNK],
                    start=True, stop=True,
                )
                if ci_ < 12 and ci_ % 2 == 1:
                    nc.vector.tensor_scalar(
                        y8[:, n0 : n0 + CHUNK], ps[:], biast, None,
                        mybir.AluOpType.add,
                    )
                else:
                    nc.scalar.activation(
                        y8[:, n0 : n0 + CHUNK], ps[:],
                        mybir.ActivationFunctionType.Identity,
                        bias=biast, scale=1.0,
                    )

        def reduce_group(c0, gc):
            """Fold chunks [c0, c0+gc) of both pacc halves into outt[:, c0:c0+gc]."""
            ps_a = psred.tile([128, gc * 128], DT_ACC, tag="ps_a")
            ps_b = psred.tile([128, gc * 128], DT_ACC, tag="ps_b")
            pst = [ps_a, ps_b]
            for half in range(2):
                for j in range(gc):
                    nc.tensor.transpose(
                        pst[half][:, j * 128 : (j + 1) * 128],
                        paccs[half][:, (c0 + j) * 128 : (c0 + j + 1) * 128],
                        ident[:],
                    )
            pt = redpool.tile([128, 2 * GROUP * 128], DT_ACC, tag="PT")
            for half in range(2):
                nc.scalar.copy(
                    pt[:, half * gc * 128 : (half + 1) * gc * 128], pst[half][:]
                )
            # pt layout: [p][s=2][c=gc][ci=8][co=16] (valid region only)
            v = pt[:, 0 : 2 * gc * 128].rearrange(
                "p (s c ci co) -> p s c ci co", s=2, c=gc, ci=8
            )
            nc.vector.tensor_tensor(
                v[:, :, :, 0:4, :], v[:, :, :, 0:4, :], v[:, :, :, 4:8, :], mx)
            nc.vector.tensor_tensor(
                v[:, :, :, 0:2, :], v[:, :, :, 0:2, :], v[:, :, :, 2:4, :], mx)
            nc.vector.tensor_tensor(
                v[:, :, :, 0:1, :], v[:, :, :, 0:1, :], v[:, :, :, 1:2, :], mx)
            # min(A-half, B-half) -> co 0..15 vs 16..31
            nc.vector.tensor_tensor(
                v[:, 0:1, :, 0:1, :], v[:, 0:1, :, 0:1, :], v[:, 1:2, :, 0:1, :], mn)
            w2 = v[:, 0, :, 0, :]  # [p, c, co16]
            nc.vector.tensor_reduce(
                outt[:, c0 : c0 + gc], w2, mybir.AxisListType.X, mn)

        # --- step 2, banded; reduce band k while band k+1 computes ---
        # conv pieces 4-7 are issued between band 0 and band 1 so that ACT
        # serves band-0 products before the remaining evacuations.
        def band(h0, h1):
            bh = h1 - h0
            # tap 0 initializes acc over the full band (incl. junk cols 126/127);
            # band 0 splits the init so DVE starts before all its rows are ready
            splits = [(h0, 21), (h0 + 21, bh - 21)] if h0 == 0 else [(h0, bh)]
            for s0, sh in splits:
                for half in range(2):
                    nc.vector.tensor_scalar(
                        _r3(paccs[half], s0, sh, 0, W), _r3(y8, s0, sh, 0, W),
                        scalt[:, half * 9 : half * 9 + 1], None, mult,
                    )
            # taps with kw=1 read y8 at a 2-byte-misaligned column offset,
            # which drops DVE tensor_scalar out of its packed modes -> ACT.
            # ACT additionally takes taps 2,5 (kw=2) to balance engine load.
            for t in range(1, 9):
                kh, kw = TAPS[t]
                nh = min(h1, OH - kh) - h0
                nw = OW - kw
                prods = []
                for half in range(2):
                    on_act = t in (1, 2, 4, 5, 7) or (
                        t == 8 and half == 0 and h0 >= 42)
                    pool = ppa if on_act else ppv
                    p = pool.tile([128, 42 * W], DT_ACC, tag="PA" if on_act else "PV")
                    p3 = p[:].rearrange("p (h w) -> p h w", w=W)[:, 0:nh, 0:nw]
                    src3 = _r3(y8, h0 + kh, nh, kw, nw)
                    sc = scalt[:, half * 9 + t : half * 9 + t + 1]
                    if on_act:
                        nc.scalar.mul(p3, src3, sc)
                    else:
                        nc.vector.tensor_scalar(p3, src3, sc, None, mult)
                    prods.append(p3)
                for half in range(2):
                    acc3 = _r3(paccs[half], h0, nh, 0, nw)
                    nc.vector.tensor_tensor(acc3, acc3, prods[half], mx)
            g = 10 if h1 == OH else GROUP  # finer groups in the last band
            for c0 in range(h0, h1, g):
                reduce_group(c0, min(g, h1 - c0))

        for pc in range(4):
            conv_piece(pc)
        band(*BANDS[0])
        for pc in range(4, 8):
            conv_piece(pc)
        for b in BANDS[1:]:
            band(*b)

        # transpose OUT[w,h] -> [h,w] and write out
        pso = psred.tile([128, 128], F32, tag="pso")
        nc.tensor.transpose(pso[0:OH, :], outt[:, 0:OH], ident_f32[:])
        res = consts.tile([128, 128], F32)
        nc.scalar.copy(res[0:OH, :], pso[0:OH, :])
        nc.sync.dma_start(out_d[:, :], res[0:OH, 0:OW])

    nc.compile()
    return nc


def host_tiles(weight, bias):
    weight = np.asarray(weight, np.float32)
    bias = np.asarray(bias, np.float32)
    w1rep = np.zeros((72, 128), np.float32)
    for kh in range(K):
        for kw in range(K):
            for ci_in in range(CIN):
                t = (kh * K + kw) * CIN + ci_in
                for ci_out in range(CIN):
                    w1rep[t, ci_out * 16 : ci_out * 16 + 16] = weight[
                        ci_out, ci_in, kh, kw
                    ]
    bias128 = np.repeat(bias[:CIN], 16).astype(np.float32).reshape(128, 1)
    scal = np.zeros((128, 18), np.float32)
    for p in range(128):
        ci = p // 16
        co_lo = p % 16
        for half in range(2):
            co = co_lo + 16 * half
            for t, (kh, kw) in enumerate(TAPS):
                scal[p, half * 9 + t] = 2.0 * weight[co, ci, kh, kw]
    consts = np.zeros((128, 19), np.float32)
    consts[:, 0:18] = scal
    consts[:, 18:19] = bias128
    return consts, w1rep.astype(ml_dtypes.bfloat16)


def im2col_host(xb):
    """xb: (8,128,128) f32 -> (72, NPIX) bf16 with junk tail cols zeroed."""
    x72 = np.zeros((72, NPIX), np.float32)
    L = NPIX - 2
    flat = xb.reshape(-1)
    for kh in range(K):
        for kw in range(K):
            for ci in range(CIN):
                t = (kh * K + kw) * CIN + ci
                off = kh * W + kw
                x72[t, :L] = flat[ci * H * W + off : ci * H * W + off + L]
    return x72.astype(ml_dtypes.bfloat16)


_CACHE = {}


def _get_program():
    if "nc" not in _CACHE:
        _CACHE["nc"] = build_program()
    return _CACHE["nc"]


def run_spmd(x, weight, bias, **kw):
    x = np.ascontiguousarray(np.asarray(x, np.float32))
    consts, w1rep = host_tiles(weight, bias)
    nc = _get_program()
    in_maps = [
        {"x72": im2col_host(x[b]), "consts": consts, "w1": w1rep}
        for b in range(NCORES)
    ]
    res = run_bass_kernel_spmd(nc, in_maps, list(range(NCORES)), **kw)
    out = np.stack([res.results[b]["out"] for b in range(NCORES)])
    return out[:, None, :, :].astype(np.float32), res


def kernel(x, weight, bias):
    out, _ = run_spmd(x, weight, bias)
    return out


if __name__ == "__main__":
    rng = np.random.default_rng(0)
    x = rng.standard_normal((8, CIN, H, W), dtype=np.float32)
    wt = rng.uniform(-0.1, 0.1, (COUT, CIN, K, K)).astype(np.float32)
    bs = rng.uniform(-0.1, 0.1, COUT).astype(np.float32)
    print(kernel(x, wt, bs).shape)
